# revision 37
# baseline (speedup 1.0000x reference)
"""Bezier-to-image Gaussian splat kernel for Trainium2 (8 NeuronCores).

Reference computation (per sample b of 256):
    T = warped cubic Bernstein basis (30, 4)
    points = einsum('nk,blkc->blnc', T, x.reshape(B,160,4,2))   # (B,160,30,2)
    gx[b,l,i,n] = exp(-(i/60 - X[b,l,n])^2 / 2e-4)
    out[b,i,j]  = min(sum_{l,n} gx[b,l,i,n]*gy[b,l,j,n], 1)     # (B,60,60)

Strategy: pure data parallel, 32 samples per core.  The host pre-transposes
control points into a [20, 2560] layout (4 curve-strips x (4 ctrl rows +
ones row)) so the whole input is ONE contiguous DMA, and a single
block-diagonal [20,128] stationary computes r256 = round(256*60*X) for a
PAIR of samples per matmul.  The banded distance d256 = 256*i - r256 is an
all-int16 packed tensor_tensor (DVE 2x mode: every operand must be
2-byte with a stride-1 count>=2 innermost AP dim — hence r is stored
duplicated x2); the Gaussian is ONE Derivative_Erf activation per group
(ACT cost is free-size only; strided ACT APs measured 5.3x slower, so
ACT reads and writes flat and the subtract itself emits the
chunk-blocked layout the 60x60 PSUM accumulation matmuls consume).
ACT is the roofline engine (~133us busy); the group schedule (a split
first sample, singles at both ends, pairs in the middle) plus 3-deep
dd/gg rings keeps it >98% occupied between first and last LUT.
Measured: 272.6us (prior baseline) -> 155.9us, rel err 4.7e-3.
"""

import math

import numpy as np
import orjson

import bass_rust
import concourse.bass as bass
import concourse.mybir as mybir
import concourse.tile as tile
from concourse.bass_utils import run_bass_kernel_spmd

B, L, N, W = 256, 160, 30, 60
NCORES = 8
BC = B // NCORES          # samples per core (32)
NPAIR = BC // 2           # 16
ALPHA = 2e-4
KEXP = 1.0 / (W * W * ALPHA)          # exponent scale in cell units: 1/0.72
SDERF = math.sqrt(KEXP)               # Derivative_Erf input scale (per cell)
DERF_FIX = math.pi / 4.0              # undo (2/sqrt(pi))^2 from Derivative_Erf
CHUNKS = 40                           # 4 curves x 30 samples per chunk
PTS = 128                             # chunk partition dim: p = 32*lg + n
CW = 60                               # width of one chunk's band (= W)
R_HOLE = -15360.0                     # r256 for dead rows -> d256 large -> g=0
Q = 256.0                             # fixed-point scale (1/256 cell)

LAST_RESULTS = None  # test harness reads profiling info from here


def _basis_T() -> np.ndarray:
    t = np.arange(N, dtype=np.float32) / np.float32(N)
    t = 2 * t**3 - 3 * t**2 + 2 * t
    t_3_0 = t**3
    t_2_1 = t**2 - t_3_0
    t_1_2 = t_3_0 - 2 * t**2 + t
    t_0_3 = (1 - t) ** 3
    return np.stack([t_3_0, 3 * t_2_1, 3 * t_1_2, t_0_3], axis=1).astype(np.float32)


def _legalize_waits(nc, max_waits: int = 1):
    """Walrus rejects engine instructions carrying more than ~1 sync wait
    ("Too many sync wait commands").  Hoist excess waits onto same-engine
    Drain instructions inserted immediately before the offender."""
    js = orjson.loads(mybir.module_to_json_bytes(nc.m))
    ctr = 0
    for f in js["functions"]:
        for bb in f["blocks"]:
            out = []
            changed = False
            for inst in bb["instructions"]:
                si = inst.get("sync_info")
                waits = si.get("on_wait") if si else None
                if waits and len(waits) > max_waits:
                    keep = waits[:max_waits]
                    for w in waits[max_waits:]:
                        ctr += 1
                        out.append({
                            "debug": inst.get("debug", 0),
                            "engine": inst["engine"],
                            "ins": [], "outs": [],
                            "name": f"waitfix-{ctr}",
                            "opcode": "Drain",
                            "sync_info": {"on_update": [], "on_wait": [w]},
                        })
                    si["on_wait"] = keep
                    changed = True
                out.append(inst)
            if changed:
                bb["instructions"] = out
    if ctr:
        nc.m = bass_rust.module_from_json_bytes(orjson.dumps(js))
    return ctr


def _host_ctrl(x_core: np.ndarray) -> np.ndarray:
    """[32,160,8] f32 -> [20, 2560] f32: row 5*lg+k col (b,c,cc) =
    x[b, 4c+lg, 2k+cc] for k<4; row 5*lg+4 = 1.0 (ones row for holes)."""
    xr = x_core.reshape(BC, CHUNKS, 4, 4, 2)          # b, c, lg, k, cc
    arr = np.ones((4, 5, BC, CHUNKS, 2), dtype=np.float32)
    arr[:, :4] = xr.transpose(2, 3, 0, 1, 4)          # lg, k, b, c, cc
    return np.ascontiguousarray(arr.reshape(20, BC * CHUNKS * 2))


def build_program(legalize: bool = True):
    f32 = mybir.dt.float32
    f16 = mybir.dt.float16
    i16 = mybir.dt.int16

    nc = bass.Bass("TRN2", target_bir_lowering=False, debug=False)

    x_t = nc.dram_tensor("x", [20, BC * CHUNKS * 2], f32, kind="ExternalInput")
    y_t = nc.dram_tensor("y", [BC, W, W], f32, kind="ExternalOutput")

    # Block-diagonal stationary: col m = 32*lg + n gets 256*60*T[n,k] from
    # row 5*lg+k; hole cols n in {30,31} get R_HOLE via the ones row 5*lg+4.
    tsc_np = np.zeros((20, 128), dtype=np.float32)
    Tb = (Q * W) * _basis_T()                         # (30, 4)
    for lg in range(4):
        tsc_np[5 * lg : 5 * lg + 4, 32 * lg : 32 * lg + 30] = Tb.T
        tsc_np[5 * lg + 4, 32 * lg + 30 : 32 * lg + 32] = R_HOLE
    tsc_d = nc.inline_tensor(tsc_np, name="tscT")

    # iota: value 256*w at offset w: [128, 60] int16
    iota_np = np.tile((Q * np.arange(CW)).astype(np.int16)[None, :], (PTS, 1))
    iota_d = nc.inline_tensor(iota_np, name="iota256")

    PAIR_F = 2 * CHUNKS * CW * 3                      # 14400 band elems: 3 samples
    SAMP_F = 2 * CHUNKS * CW                          # 4800 per sample
    # input DMA column slices (80 cols = 1 sample): small first slices so
    # the first r-matmul fires as early as possible
    SLICES = [(0, 160), (160, 320), (320, 480), (480, 640)] + [
        (640 + 320 * k, 960 + 320 * k) for k in range(6)
    ]
    # ACT groups as band segments (sample, lo, hi): sample 0 is split into
    # quarter + rest so the first LUT fires after a 0.7us quarter-subtract;
    # the middle runs as triples (largest batch whose 3-deep tile ring
    # still fits SBUF) to amortize the ~290ns per-activation init; singles
    # at the end shorten the tail.  `done` = samples completed this group.
    H = SAMP_F // 4
    GROUPS = (
        [
            {"segs": [(0, 0, H)], "done": []},
            {"segs": [(0, H, SAMP_F)], "done": [0]},
            {"segs": [(1, 0, SAMP_F)], "done": [1]},
            {"segs": [(2, 0, SAMP_F)], "done": [2]},
            {"segs": [(3, 0, SAMP_F)], "done": [3]},
        ]
        + [
            {"segs": [(3 * t + 4, 0, SAMP_F), (3 * t + 5, 0, SAMP_F),
                      (3 * t + 6, 0, SAMP_F)],
             "done": [3 * t + 4, 3 * t + 5, 3 * t + 6]}
            for t in range(9)
        ]
        + [
            {"segs": [(31, 0, SAMP_F)], "done": [31]},
        ]
    )

    with tile.TileContext(nc) as tc, tc.tile_pool(name="const", bufs=1) as cpool, \
            tc.tile_pool(name="ctrl", bufs=1) as ctrl_pool, \
            tc.tile_pool(name="outp", bufs=2) as out_pool, \
            tc.tile_pool(name="dd", bufs=3) as dd_pool, \
            tc.tile_pool(name="gg", bufs=3) as gg_pool, \
            tc.tile_pool(name="rps", bufs=3, space="PSUM") as rps_pool, \
            tc.tile_pool(name="img", bufs=4, space="PSUM") as img_pool:

        # issue order matters: the SP sequencer generates DMA descriptors
        # serially (~0.9us each), so the first ctrl slice goes first — it
        # gates the whole pipeline ramp.
        cts = []
        c0, c1 = SLICES[0]
        ct_s = ctrl_pool.tile([20, c1 - c0], f32, tag=f"ct{c0}")
        nc.sync.dma_start(ct_s[:], x_t.ap()[:, c0:c1])
        cts.append((c0, c1, ct_s))

        tsc = cpool.tile([20, 128], f32, tag="tsc")
        nc.sync.dma_start(tsc[:], tsc_d.ap())
        iot = cpool.tile([PTS, CW], i16, tag="iota")
        nc.sync.dma_start(iot[:], iota_d.ap())

        for c0, c1 in SLICES[1:]:
            ct_s = ctrl_pool.tile([20, c1 - c0], f32, tag=f"ct{c0}")
            nc.sync.dma_start(ct_s[:], x_t.ap()[:, c0:c1])
            cts.append((c0, c1, ct_s))

        def ct_slice(P):
            """AP of the 160 ctrl columns of pair P inside its slice tile."""
            for c0, c1, t in cts:
                if c0 <= 160 * P and 160 * (P + 1) <= c1:
                    return t[:, 160 * P - c0 : 160 * (P + 1) - c0]
            raise AssertionError(P)

        # r256 for all 16 pairs, each value duplicated x2 so the banded
        # subtract can keep a packed (stride-1, count-2) innermost dim on
        # the r operand while its output walks the chunk-blocked band
        # contiguously (DVE 2x mode needs every operand packed innermost).
        r_all = ctrl_pool.tile([PTS, NPAIR * 320], i16, tag="rall")

        NG = len(GROUPS)
        dd_t = [None] * NG
        gg_t = [None] * NG
        img_t = [None] * BC
        seg_map = {}   # sample -> list of (group, pos_in_tile, lo, hi)

        def emit_rmm(P):
            r_ps = rps_pool.tile([PTS, 160], f32, tag="rps")
            nc.tensor.matmul(
                r_ps[:], lhsT=tsc[:], rhs=ct_slice(P),
                start=True, stop=True,
            )
            nc.vector.tensor_copy(
                r_all[:, 320 * P : 320 * P + 320].rearrange(
                    "p (cs d) -> p cs d", d=2
                ),
                r_ps[:].rearrange("p (cs o) -> p cs o", o=1)
                .broadcast_to([PTS, 160, 2]),
            )

        next_rmm = [0]

        def ensure_rmm(g):
            """Emit r matmuls for all pairs group g touches."""
            if g >= NG:
                return
            need = max(b // 2 for b, _, _ in GROUPS[g]["segs"])
            while next_rmm[0] <= need:
                emit_rmm(next_rmm[0])
                next_rmm[0] += 1

        def emit_sub(g):
            dd = dd_pool.tile([PTS, PAIR_F], i16, tag="dd")
            dd_t[g] = dd
            pos = 0
            for b, lo, hi in GROUPS[g]["segs"]:
                seg_map.setdefault(b, []).append((g, pos, lo, hi))
                roff = 320 * (b // 2) + 160 * (b % 2)
                ncs = (hi - lo) // CW
                # d256[p, (cs, w)] = 256*w - r256[p, cs], chunk-blocked out.
                # Iteration (cs, w_hi, w_lo=2): out/iota walk contiguously,
                # r reads its duplicated pair -> all operands packed -> 2x.
                nc.vector.tensor_tensor(
                    dd[:, pos : pos + hi - lo].rearrange(
                        "p (cs wh wl) -> p cs wh wl", cs=ncs, wl=2
                    ),
                    iot[:].rearrange("p (o wh wl) -> p o wh wl", o=1, wl=2)
                    .broadcast_to([PTS, ncs, CW // 2, 2]),
                    r_all[:, roff + 2 * (lo // CW) : roff + 2 * (hi // CW)]
                    .rearrange("p (cs o wl) -> p cs o wl", o=1, wl=2)
                    .broadcast_to([PTS, ncs, CW // 2, 2]),
                    mybir.AluOpType.subtract,
                )
                pos += hi - lo

        def emit_act(g):
            # gg keeps dd's flat layout: ACT reads AND writes fully packed
            # 1-D (a permuted/strided ACT output AP measured 5.3x slower on
            # HW); the chunk-blocked layout comes from the subtract itself.
            n = sum(hi - lo for _, lo, hi in GROUPS[g]["segs"])
            gg = gg_pool.tile([PTS, PAIR_F], f16, tag="gg")
            gg_t[g] = gg
            dd = dd_t[g]
            nc.scalar.activation(
                gg[:, :n], dd[:, :n],
                mybir.ActivationFunctionType.Derivative_Erf,
                bias=0.0, scale=SDERF / Q,
            )

        def gg_at(b, off):
            """(tile, col) holding band element `off` of sample b."""
            for g, pos, lo, hi in seg_map[b]:
                if lo <= off < hi:
                    return gg_t[g], pos + off - lo
            raise AssertionError((b, off))

        def emit_img(g):
            for b in GROUPS[g]["done"]:
                img = img_pool.tile([W, W], f32, tag="img")
                img_t[b] = img
                for c in range(CHUNKS):
                    gx, ox = gg_at(b, 2 * CW * c)
                    gy, oy = gg_at(b, 2 * CW * c + CW)
                    nc.tensor.matmul(
                        img[:],
                        lhsT=gx[:, ox : ox + W],
                        rhs=gy[:, oy : oy + W],
                        start=(c == 0),
                        stop=(c == CHUNKS - 1),
                    )

        def emit_min_store(g):
            done = GROUPS[g]["done"]
            if not done:
                return
            n = len(done)
            outp = out_pool.tile([W, 3 * W], f32, tag="op")
            for k, b in enumerate(done):
                nc.vector.tensor_scalar(
                    outp[:, W * k : W * (k + 1)],
                    img_t[b][:],
                    DERF_FIX, 1.0,
                    mybir.AluOpType.mult, mybir.AluOpType.min,
                )
            nc.sync.dma_start(
                y_t.ap()[done[0] : done[0] + n].rearrange("b i j -> i b j"),
                outp[:, : W * n].rearrange("i (b j) -> i b j", b=n),
            )

        # -------- software-pipelined emission --------
        ensure_rmm(0)
        ensure_rmm(1)
        ensure_rmm(2)
        emit_sub(0)
        for g in range(NG):
            ensure_rmm(g + 3)
            if g + 1 < NG:
                emit_sub(g + 1)
            emit_act(g)
            emit_img(g)
            if g >= 1:
                emit_min_store(g - 1)
        emit_min_store(NG - 1)

    if legalize:
        _legalize_waits(nc)
    return nc


_PROGRAM = None


def kernel(x: np.ndarray, _trace: bool = False) -> np.ndarray:
    global _PROGRAM, LAST_RESULTS
    assert x.shape == (B, L, 8) and x.dtype == np.float32, (x.shape, x.dtype)
    if _PROGRAM is None:
        _PROGRAM = build_program()
    nc = _PROGRAM
    shards = np.split(np.ascontiguousarray(x), NCORES, axis=0)
    in_maps = [{"x": _host_ctrl(s)} for s in shards]
    res = run_bass_kernel_spmd(nc, in_maps, list(range(NCORES)), trace=_trace)
    LAST_RESULTS = res
    return np.concatenate([res.results[i]["y"] for i in range(NCORES)], axis=0)


# revision 38
# speedup vs baseline: 1.0166x; 1.0166x over previous
"""Bezier-to-image Gaussian splat kernel for Trainium2 (8 NeuronCores).

Reference computation (per sample b of 256):
    T = warped cubic Bernstein basis (30, 4)
    points = einsum('nk,blkc->blnc', T, x.reshape(B,160,4,2))   # (B,160,30,2)
    gx[b,l,i,n] = exp(-(i/60 - X[b,l,n])^2 / 2e-4)
    out[b,i,j]  = min(sum_{l,n} gx[b,l,i,n]*gy[b,l,j,n], 1)     # (B,60,60)

Strategy: pure data parallel, 32 samples per core.  The host pre-transposes
control points into a [20, 2560] layout (4 curve-strips x (4 ctrl rows +
ones row)) so the whole input is ONE contiguous DMA, and a single
block-diagonal [20,128] stationary computes r256 = round(256*60*X) for a
PAIR of samples per matmul.  The banded distance d256 = 256*i - r256 is an
all-int16 packed tensor_tensor (DVE 2x mode: every operand must be
2-byte with a stride-1 count>=2 innermost AP dim — hence r is stored
duplicated x2); the Gaussian is ONE Derivative_Erf activation per group
(ACT cost is free-size only; strided ACT APs measured 5.3x slower, so
ACT reads and writes flat and the subtract itself emits the
chunk-blocked layout the 60x60 PSUM accumulation matmuls consume).
ACT is the roofline engine (~133us busy); the group schedule (a split
first sample, singles at both ends, pairs in the middle) plus 3-deep
dd/gg rings keeps it >98% occupied between first and last LUT.
Measured: 272.6us (prior baseline) -> 155.9us, rel err 4.7e-3.
"""

import math

import numpy as np
import orjson

import bass_rust
import concourse.bass as bass
import concourse.mybir as mybir
import concourse.tile as tile
from concourse.bass_utils import run_bass_kernel_spmd

B, L, N, W = 256, 160, 30, 60
NCORES = 8
BC = B // NCORES          # samples per core (32)
NPAIR = BC // 2           # 16
ALPHA = 2e-4
KEXP = 1.0 / (W * W * ALPHA)          # exponent scale in cell units: 1/0.72
SDERF = math.sqrt(KEXP)               # Derivative_Erf input scale (per cell)
DERF_FIX = math.pi / 4.0              # undo (2/sqrt(pi))^2 from Derivative_Erf
CHUNKS = 40                           # 4 curves x 30 samples per chunk
PTS = 128                             # chunk partition dim: p = 32*lg + n
CW = 60                               # width of one chunk's band (= W)
R_HOLE = -15360.0                     # r256 for dead rows -> d256 large -> g=0
Q = 256.0                             # fixed-point scale (1/256 cell)

LAST_RESULTS = None  # test harness reads profiling info from here


def _basis_T() -> np.ndarray:
    t = np.arange(N, dtype=np.float32) / np.float32(N)
    t = 2 * t**3 - 3 * t**2 + 2 * t
    t_3_0 = t**3
    t_2_1 = t**2 - t_3_0
    t_1_2 = t_3_0 - 2 * t**2 + t
    t_0_3 = (1 - t) ** 3
    return np.stack([t_3_0, 3 * t_2_1, 3 * t_1_2, t_0_3], axis=1).astype(np.float32)


def _legalize_waits(nc, max_waits: int = 1):
    """Walrus rejects engine instructions carrying more than ~1 sync wait
    ("Too many sync wait commands").  Hoist excess waits onto same-engine
    Drain instructions inserted immediately before the offender."""
    js = orjson.loads(mybir.module_to_json_bytes(nc.m))
    ctr = 0
    for f in js["functions"]:
        for bb in f["blocks"]:
            out = []
            changed = False
            for inst in bb["instructions"]:
                si = inst.get("sync_info")
                waits = si.get("on_wait") if si else None
                if waits and len(waits) > max_waits:
                    keep = waits[:max_waits]
                    for w in waits[max_waits:]:
                        ctr += 1
                        out.append({
                            "debug": inst.get("debug", 0),
                            "engine": inst["engine"],
                            "ins": [], "outs": [],
                            "name": f"waitfix-{ctr}",
                            "opcode": "Drain",
                            "sync_info": {"on_update": [], "on_wait": [w]},
                        })
                    si["on_wait"] = keep
                    changed = True
                out.append(inst)
            if changed:
                bb["instructions"] = out
    if ctr:
        nc.m = bass_rust.module_from_json_bytes(orjson.dumps(js))
    return ctr


def _host_ctrl(x_core: np.ndarray) -> np.ndarray:
    """[32,160,8] f32 -> [20, 2560] f32: row 5*lg+k col (b,c,cc) =
    x[b, 4c+lg, 2k+cc] for k<4; row 5*lg+4 = 1.0 (ones row for holes)."""
    xr = x_core.reshape(BC, CHUNKS, 4, 4, 2)          # b, c, lg, k, cc
    arr = np.ones((4, 5, BC, CHUNKS, 2), dtype=np.float32)
    arr[:, :4] = xr.transpose(2, 3, 0, 1, 4)          # lg, k, b, c, cc
    return np.ascontiguousarray(arr.reshape(20, BC * CHUNKS * 2))


def build_program(legalize: bool = True):
    f32 = mybir.dt.float32
    f16 = mybir.dt.float16
    i16 = mybir.dt.int16

    nc = bass.Bass("TRN2", target_bir_lowering=False, debug=False)

    x_t = nc.dram_tensor("x", [20, BC * CHUNKS * 2], f32, kind="ExternalInput")
    y_t = nc.dram_tensor("y", [BC, W, W], f32, kind="ExternalOutput")

    # Block-diagonal stationary: col m = 32*lg + n gets 256*60*T[n,k] from
    # row 5*lg+k; hole cols n in {30,31} get R_HOLE via the ones row 5*lg+4.
    tsc_np = np.zeros((20, 128), dtype=np.float32)
    Tb = (Q * W) * _basis_T()                         # (30, 4)
    for lg in range(4):
        tsc_np[5 * lg : 5 * lg + 4, 32 * lg : 32 * lg + 30] = Tb.T
        tsc_np[5 * lg + 4, 32 * lg + 30 : 32 * lg + 32] = R_HOLE
    tsc_d = nc.inline_tensor(tsc_np, name="tscT")

    # iota: value 256*w at offset w: [128, 60] int16
    iota_np = np.tile((Q * np.arange(CW)).astype(np.int16)[None, :], (PTS, 1))
    iota_d = nc.inline_tensor(iota_np, name="iota256")

    PAIR_F = 2 * CHUNKS * CW * 2                      # 9600 band elems per pair
    SAMP_F = 2 * CHUNKS * CW                          # 4800 per sample
    # input DMA column slices (80 cols = 1 sample): small first slices so
    # the first r-matmul fires as early as possible
    SLICES = [(0, 160), (160, 320), (320, 480), (480, 640)] + [
        (640 + 320 * k, 960 + 320 * k) for k in range(6)
    ]
    # ACT groups as band segments (sample, lo, hi): sample 0 is split into
    # two half bands so the first LUT fires after a 1.3us half-subtract;
    # singles at the end shorten the tail.  `done` lists samples whose band
    # completes with this group (image matmuls can then run).
    H = SAMP_F // 2
    GROUPS = (
        [
            {"segs": [(0, 0, H)], "done": []},
            {"segs": [(0, H, SAMP_F)], "done": [0]},
            {"segs": [(1, 0, SAMP_F)], "done": [1]},
            {"segs": [(2, 0, SAMP_F)], "done": [2]},
        ]
        + [
            {"segs": [(2 * p + 1, 0, SAMP_F), (2 * p + 2, 0, SAMP_F)],
             "done": [2 * p + 1, 2 * p + 2]}
            for p in range(1, 15)
        ]
        + [
            {"segs": [(31, 0, SAMP_F)], "done": [31]},
        ]
    )

    with tile.TileContext(nc) as tc, tc.tile_pool(name="const", bufs=1) as cpool, \
            tc.tile_pool(name="ctrl", bufs=1) as ctrl_pool, \
            tc.tile_pool(name="outp", bufs=2) as out_pool, \
            tc.tile_pool(name="dd", bufs=3) as dd_pool, \
            tc.tile_pool(name="gg", bufs=3) as gg_pool, \
            tc.tile_pool(name="rps", bufs=3, space="PSUM") as rps_pool, \
            tc.tile_pool(name="img", bufs=4, space="PSUM") as img_pool:

        # issue order matters: the SP sequencer generates DMA descriptors
        # serially (~0.9us each), so the first ctrl slice goes first — it
        # gates the whole pipeline ramp.
        cts = []
        c0, c1 = SLICES[0]
        ct_s = ctrl_pool.tile([20, c1 - c0], f32, tag=f"ct{c0}")
        nc.sync.dma_start(ct_s[:], x_t.ap()[:, c0:c1])
        cts.append((c0, c1, ct_s))

        tsc = cpool.tile([20, 128], f32, tag="tsc")
        nc.sync.dma_start(tsc[:], tsc_d.ap())
        iot = cpool.tile([PTS, CW], i16, tag="iota")
        nc.sync.dma_start(iot[:], iota_d.ap())

        for c0, c1 in SLICES[1:]:
            ct_s = ctrl_pool.tile([20, c1 - c0], f32, tag=f"ct{c0}")
            nc.sync.dma_start(ct_s[:], x_t.ap()[:, c0:c1])
            cts.append((c0, c1, ct_s))

        def ct_slice(P):
            """AP of the 160 ctrl columns of pair P inside its slice tile."""
            for c0, c1, t in cts:
                if c0 <= 160 * P and 160 * (P + 1) <= c1:
                    return t[:, 160 * P - c0 : 160 * (P + 1) - c0]
            raise AssertionError(P)

        # r256 for all 16 pairs, each value duplicated x2 so the banded
        # subtract can keep a packed (stride-1, count-2) innermost dim on
        # the r operand while its output walks the chunk-blocked band
        # contiguously (DVE 2x mode needs every operand packed innermost).
        r_all = ctrl_pool.tile([PTS, NPAIR * 320], i16, tag="rall")

        NG = len(GROUPS)
        dd_t = [None] * NG
        gg_t = [None] * NG
        img_t = [None] * BC
        seg_map = {}   # sample -> list of (group, pos_in_tile, lo, hi)

        def emit_rmm(P):
            r_ps = rps_pool.tile([PTS, 160], f32, tag="rps")
            nc.tensor.matmul(
                r_ps[:], lhsT=tsc[:], rhs=ct_slice(P),
                start=True, stop=True,
            )
            nc.vector.tensor_copy(
                r_all[:, 320 * P : 320 * P + 320].rearrange(
                    "p (cs d) -> p cs d", d=2
                ),
                r_ps[:].rearrange("p (cs o) -> p cs o", o=1)
                .broadcast_to([PTS, 160, 2]),
            )

        next_rmm = [0]

        def ensure_rmm(g):
            """Emit r matmuls for all pairs group g touches."""
            if g >= NG:
                return
            need = max(b // 2 for b, _, _ in GROUPS[g]["segs"])
            while next_rmm[0] <= need:
                emit_rmm(next_rmm[0])
                next_rmm[0] += 1

        def emit_sub(g):
            dd = dd_pool.tile([PTS, PAIR_F], i16, tag="dd")
            dd_t[g] = dd
            pos = 0
            for b, lo, hi in GROUPS[g]["segs"]:
                seg_map.setdefault(b, []).append((g, pos, lo, hi))
                roff = 320 * (b // 2) + 160 * (b % 2)
                ncs = (hi - lo) // CW
                # d256[p, (cs, w)] = 256*w - r256[p, cs], chunk-blocked out.
                # Iteration (cs, w_hi, w_lo=2): out/iota walk contiguously,
                # r reads its duplicated pair -> all operands packed -> 2x.
                nc.vector.tensor_tensor(
                    dd[:, pos : pos + hi - lo].rearrange(
                        "p (cs wh wl) -> p cs wh wl", cs=ncs, wl=2
                    ),
                    iot[:].rearrange("p (o wh wl) -> p o wh wl", o=1, wl=2)
                    .broadcast_to([PTS, ncs, CW // 2, 2]),
                    r_all[:, roff + 2 * (lo // CW) : roff + 2 * (hi // CW)]
                    .rearrange("p (cs o wl) -> p cs o wl", o=1, wl=2)
                    .broadcast_to([PTS, ncs, CW // 2, 2]),
                    mybir.AluOpType.subtract,
                )
                pos += hi - lo

        def emit_act(g):
            # gg keeps dd's flat layout: ACT reads AND writes fully packed
            # 1-D (a permuted/strided ACT output AP measured 5.3x slower on
            # HW); the chunk-blocked layout comes from the subtract itself.
            n = sum(hi - lo for _, lo, hi in GROUPS[g]["segs"])
            gg = gg_pool.tile([PTS, PAIR_F], f16, tag="gg")
            gg_t[g] = gg
            dd = dd_t[g]
            nc.scalar.activation(
                gg[:, :n], dd[:, :n],
                mybir.ActivationFunctionType.Derivative_Erf,
                bias=0.0, scale=SDERF / Q,
            )

        def gg_at(b, off):
            """(tile, col) holding band element `off` of sample b."""
            for g, pos, lo, hi in seg_map[b]:
                if lo <= off < hi:
                    return gg_t[g], pos + off - lo
            raise AssertionError((b, off))

        def emit_img(g):
            for b in GROUPS[g]["done"]:
                img = img_pool.tile([W, W], f32, tag="img")
                img_t[b] = img
                for c in range(CHUNKS):
                    gx, ox = gg_at(b, 2 * CW * c)
                    gy, oy = gg_at(b, 2 * CW * c + CW)
                    nc.tensor.matmul(
                        img[:],
                        lhsT=gx[:, ox : ox + W],
                        rhs=gy[:, oy : oy + W],
                        start=(c == 0),
                        stop=(c == CHUNKS - 1),
                    )

        def emit_min_store(g):
            done = GROUPS[g]["done"]
            if not done:
                return
            n = len(done)
            outp = out_pool.tile([W, 2 * W], f32, tag="op")
            for k, b in enumerate(done):
                nc.vector.tensor_scalar(
                    outp[:, W * k : W * (k + 1)],
                    img_t[b][:],
                    DERF_FIX, 1.0,
                    mybir.AluOpType.mult, mybir.AluOpType.min,
                )
            nc.sync.dma_start(
                y_t.ap()[done[0] : done[0] + n].rearrange("b i j -> i b j"),
                outp[:, : W * n].rearrange("i (b j) -> i b j", b=n),
            )

        # -------- software-pipelined emission --------
        ensure_rmm(0)
        ensure_rmm(1)
        ensure_rmm(2)
        emit_sub(0)
        for g in range(NG):
            ensure_rmm(g + 3)
            if g + 1 < NG:
                emit_sub(g + 1)
            emit_act(g)
            emit_img(g)
            if g >= 1:
                emit_min_store(g - 1)
        emit_min_store(NG - 1)

    if legalize:
        _legalize_waits(nc)
    return nc


_PROGRAM = None


def kernel(x: np.ndarray, _trace: bool = False) -> np.ndarray:
    global _PROGRAM, LAST_RESULTS
    assert x.shape == (B, L, 8) and x.dtype == np.float32, (x.shape, x.dtype)
    if _PROGRAM is None:
        _PROGRAM = build_program()
    nc = _PROGRAM
    shards = np.split(np.ascontiguousarray(x), NCORES, axis=0)
    in_maps = [{"x": _host_ctrl(s)} for s in shards]
    res = run_bass_kernel_spmd(nc, in_maps, list(range(NCORES)), trace=_trace)
    LAST_RESULTS = res
    return np.concatenate([res.results[i]["y"] for i in range(NCORES)], axis=0)


# revision 39
# speedup vs baseline: 1.0203x; 1.0035x over previous
"""Bezier-to-image Gaussian splat kernel for Trainium2 (8 NeuronCores).

Reference computation (per sample b of 256):
    T = warped cubic Bernstein basis (30, 4)
    points = einsum('nk,blkc->blnc', T, x.reshape(B,160,4,2))   # (B,160,30,2)
    gx[b,l,i,n] = exp(-(i/60 - X[b,l,n])^2 / 2e-4)
    out[b,i,j]  = min(sum_{l,n} gx[b,l,i,n]*gy[b,l,j,n], 1)     # (B,60,60)

Strategy: pure data parallel, 32 samples per core.  The host pre-transposes
control points into a [20, 2560] layout (4 curve-strips x (4 ctrl rows +
ones row)) so the whole input is ONE contiguous DMA, and a single
block-diagonal [20,128] stationary computes r256 = round(256*60*X) for a
PAIR of samples per matmul.  The banded distance d256 = 256*i - r256 is an
all-int16 packed tensor_tensor (DVE 2x mode: every operand must be
2-byte with a stride-1 count>=2 innermost AP dim — hence r is stored
duplicated x2); the Gaussian is ONE Derivative_Erf activation per group
(ACT cost is free-size only; strided ACT APs measured 5.3x slower, so
ACT reads and writes flat and the subtract itself emits the
chunk-blocked layout the 60x60 PSUM accumulation matmuls consume).
ACT is the roofline engine (~133us busy); the group schedule (a split
first sample, singles at both ends, pairs in the middle) plus 3-deep
dd/gg rings keeps it >98% occupied between first and last LUT.
Measured: 272.6us (prior baseline) -> 155.9us, rel err 4.7e-3.
"""

import math

import numpy as np
import orjson

import bass_rust
import concourse.bass as bass
import concourse.mybir as mybir
import concourse.tile as tile
from concourse.bass_utils import run_bass_kernel_spmd

B, L, N, W = 256, 160, 30, 60
NCORES = 8
BC = B // NCORES          # samples per core (32)
NPAIR = BC // 2           # 16
ALPHA = 2e-4
KEXP = 1.0 / (W * W * ALPHA)          # exponent scale in cell units: 1/0.72
SDERF = math.sqrt(KEXP)               # Derivative_Erf input scale (per cell)
DERF_FIX = math.pi / 4.0              # undo (2/sqrt(pi))^2 from Derivative_Erf
CHUNKS = 40                           # 4 curves x 30 samples per chunk
PTS = 128                             # chunk partition dim: p = 32*lg + n
CW = 60                               # width of one chunk's band (= W)
R_HOLE = -15360.0                     # r256 for dead rows -> d256 large -> g=0
Q = 256.0                             # fixed-point scale (1/256 cell)

LAST_RESULTS = None  # test harness reads profiling info from here


def _basis_T() -> np.ndarray:
    t = np.arange(N, dtype=np.float32) / np.float32(N)
    t = 2 * t**3 - 3 * t**2 + 2 * t
    t_3_0 = t**3
    t_2_1 = t**2 - t_3_0
    t_1_2 = t_3_0 - 2 * t**2 + t
    t_0_3 = (1 - t) ** 3
    return np.stack([t_3_0, 3 * t_2_1, 3 * t_1_2, t_0_3], axis=1).astype(np.float32)


def _legalize_waits(nc, max_waits: int = 1):
    """Walrus rejects engine instructions carrying more than ~1 sync wait
    ("Too many sync wait commands").  Hoist excess waits onto same-engine
    Drain instructions inserted immediately before the offender."""
    js = orjson.loads(mybir.module_to_json_bytes(nc.m))
    ctr = 0
    for f in js["functions"]:
        for bb in f["blocks"]:
            out = []
            changed = False
            for inst in bb["instructions"]:
                si = inst.get("sync_info")
                waits = si.get("on_wait") if si else None
                if waits and len(waits) > max_waits:
                    keep = waits[:max_waits]
                    for w in waits[max_waits:]:
                        ctr += 1
                        out.append({
                            "debug": inst.get("debug", 0),
                            "engine": inst["engine"],
                            "ins": [], "outs": [],
                            "name": f"waitfix-{ctr}",
                            "opcode": "Drain",
                            "sync_info": {"on_update": [], "on_wait": [w]},
                        })
                    si["on_wait"] = keep
                    changed = True
                out.append(inst)
            if changed:
                bb["instructions"] = out
    if ctr:
        nc.m = bass_rust.module_from_json_bytes(orjson.dumps(js))
    return ctr


def _host_ctrl(x_core: np.ndarray) -> np.ndarray:
    """[32,160,8] f32 -> [20, 2560] f32: row 5*lg+k col (b,c,cc) =
    x[b, 4c+lg, 2k+cc] for k<4; row 5*lg+4 = 1.0 (ones row for holes)."""
    xr = x_core.reshape(BC, CHUNKS, 4, 4, 2)          # b, c, lg, k, cc
    arr = np.ones((4, 5, BC, CHUNKS, 2), dtype=np.float32)
    arr[:, :4] = xr.transpose(2, 3, 0, 1, 4)          # lg, k, b, c, cc
    return np.ascontiguousarray(arr.reshape(20, BC * CHUNKS * 2))


def build_program(legalize: bool = True):
    f32 = mybir.dt.float32
    f16 = mybir.dt.float16
    i16 = mybir.dt.int16

    nc = bass.Bass("TRN2", target_bir_lowering=False, debug=False)

    x_t = nc.dram_tensor("x", [20, BC * CHUNKS * 2], f32, kind="ExternalInput")
    y_t = nc.dram_tensor("y", [BC, W, W], f32, kind="ExternalOutput")

    # Block-diagonal stationary: col m = 32*lg + n gets 256*60*T[n,k] from
    # row 5*lg+k; hole cols n in {30,31} get R_HOLE via the ones row 5*lg+4.
    tsc_np = np.zeros((20, 128), dtype=np.float32)
    Tb = (Q * W) * _basis_T()                         # (30, 4)
    for lg in range(4):
        tsc_np[5 * lg : 5 * lg + 4, 32 * lg : 32 * lg + 30] = Tb.T
        tsc_np[5 * lg + 4, 32 * lg + 30 : 32 * lg + 32] = R_HOLE
    tsc_d = nc.inline_tensor(tsc_np, name="tscT")

    # iota: value 256*w at offset w: [128, 60] int16
    iota_np = np.tile((Q * np.arange(CW)).astype(np.int16)[None, :], (PTS, 1))
    iota_d = nc.inline_tensor(iota_np, name="iota256")

    PAIR_F = 2 * CHUNKS * CW * 2                      # 9600 band elems per pair
    SAMP_F = 2 * CHUNKS * CW                          # 4800 per sample
    # input DMA column slices (80 cols = 1 sample): small first slices so
    # the first r-matmul fires as early as possible
    SLICES = [(0, 160), (160, 320), (320, 480), (480, 640)] + [
        (640 + 320 * k, 960 + 320 * k) for k in range(6)
    ]
    # ACT groups as band segments (sample, lo, hi): sample 0 is split into
    # two half bands so the first LUT fires after a 1.3us half-subtract;
    # singles at the end shorten the tail.  `done` lists samples whose band
    # completes with this group (image matmuls can then run).
    H = SAMP_F // 2
    GROUPS = (
        [
            {"segs": [(0, 0, H)], "done": []},
            {"segs": [(0, H, SAMP_F)], "done": [0]},
            {"segs": [(1, 0, SAMP_F)], "done": [1]},
            {"segs": [(2, 0, SAMP_F)], "done": [2]},
        ]
        + [
            {"segs": [(2 * p + 1, 0, SAMP_F), (2 * p + 2, 0, SAMP_F)],
             "done": [2 * p + 1, 2 * p + 2]}
            for p in range(1, 15)
        ]
        + [
            {"segs": [(31, 0, SAMP_F)], "done": [31]},
        ]
    )

    with tile.TileContext(nc) as tc, tc.tile_pool(name="const", bufs=1) as cpool, \
            tc.tile_pool(name="ctrl", bufs=1) as ctrl_pool, \
            tc.tile_pool(name="outp", bufs=2) as out_pool, \
            tc.tile_pool(name="dd", bufs=3) as dd_pool, \
            tc.tile_pool(name="gg", bufs=3) as gg_pool, \
            tc.tile_pool(name="rps", bufs=3, space="PSUM") as rps_pool, \
            tc.tile_pool(name="img", bufs=4, space="PSUM") as img_pool:

        # issue order matters: descriptor generation is serial per DGE
        # queue (~0.9us each).  The ramp-critical first ctrl slice goes on
        # the Activation engine's DGE queue (idle until the ACT table load)
        # so it runs in parallel with the SP queue doing the constants.
        cts = []
        c0, c1 = SLICES[0]
        ct_s = ctrl_pool.tile([20, c1 - c0], f32, tag=f"ct{c0}")
        nc.scalar.dma_start(ct_s[:], x_t.ap()[:, c0:c1])
        cts.append((c0, c1, ct_s))

        tsc = cpool.tile([20, 128], f32, tag="tsc")
        nc.sync.dma_start(tsc[:], tsc_d.ap())
        iot = cpool.tile([PTS, CW], i16, tag="iota")
        nc.sync.dma_start(iot[:], iota_d.ap())

        for c0, c1 in SLICES[1:]:
            ct_s = ctrl_pool.tile([20, c1 - c0], f32, tag=f"ct{c0}")
            nc.sync.dma_start(ct_s[:], x_t.ap()[:, c0:c1])
            cts.append((c0, c1, ct_s))

        def ct_slice(P):
            """AP of the 160 ctrl columns of pair P inside its slice tile."""
            for c0, c1, t in cts:
                if c0 <= 160 * P and 160 * (P + 1) <= c1:
                    return t[:, 160 * P - c0 : 160 * (P + 1) - c0]
            raise AssertionError(P)

        # r256 for all 16 pairs, each value duplicated x2 so the banded
        # subtract can keep a packed (stride-1, count-2) innermost dim on
        # the r operand while its output walks the chunk-blocked band
        # contiguously (DVE 2x mode needs every operand packed innermost).
        r_all = ctrl_pool.tile([PTS, NPAIR * 320], i16, tag="rall")

        NG = len(GROUPS)
        dd_t = [None] * NG
        gg_t = [None] * NG
        img_t = [None] * BC
        seg_map = {}   # sample -> list of (group, pos_in_tile, lo, hi)

        def emit_rmm(P):
            r_ps = rps_pool.tile([PTS, 160], f32, tag="rps")
            nc.tensor.matmul(
                r_ps[:], lhsT=tsc[:], rhs=ct_slice(P),
                start=True, stop=True,
            )
            nc.vector.tensor_copy(
                r_all[:, 320 * P : 320 * P + 320].rearrange(
                    "p (cs d) -> p cs d", d=2
                ),
                r_ps[:].rearrange("p (cs o) -> p cs o", o=1)
                .broadcast_to([PTS, 160, 2]),
            )

        next_rmm = [0]

        def ensure_rmm(g):
            """Emit r matmuls for all pairs group g touches."""
            if g >= NG:
                return
            need = max(b // 2 for b, _, _ in GROUPS[g]["segs"])
            while next_rmm[0] <= need:
                emit_rmm(next_rmm[0])
                next_rmm[0] += 1

        def emit_sub(g):
            dd = dd_pool.tile([PTS, PAIR_F], i16, tag="dd")
            dd_t[g] = dd
            pos = 0
            for b, lo, hi in GROUPS[g]["segs"]:
                seg_map.setdefault(b, []).append((g, pos, lo, hi))
                roff = 320 * (b // 2) + 160 * (b % 2)
                ncs = (hi - lo) // CW
                # d256[p, (cs, w)] = 256*w - r256[p, cs], chunk-blocked out.
                # Iteration (cs, w_hi, w_lo=2): out/iota walk contiguously,
                # r reads its duplicated pair -> all operands packed -> 2x.
                nc.vector.tensor_tensor(
                    dd[:, pos : pos + hi - lo].rearrange(
                        "p (cs wh wl) -> p cs wh wl", cs=ncs, wl=2
                    ),
                    iot[:].rearrange("p (o wh wl) -> p o wh wl", o=1, wl=2)
                    .broadcast_to([PTS, ncs, CW // 2, 2]),
                    r_all[:, roff + 2 * (lo // CW) : roff + 2 * (hi // CW)]
                    .rearrange("p (cs o wl) -> p cs o wl", o=1, wl=2)
                    .broadcast_to([PTS, ncs, CW // 2, 2]),
                    mybir.AluOpType.subtract,
                )
                pos += hi - lo

        def emit_act(g):
            # gg keeps dd's flat layout: ACT reads AND writes fully packed
            # 1-D (a permuted/strided ACT output AP measured 5.3x slower on
            # HW); the chunk-blocked layout comes from the subtract itself.
            n = sum(hi - lo for _, lo, hi in GROUPS[g]["segs"])
            gg = gg_pool.tile([PTS, PAIR_F], f16, tag="gg")
            gg_t[g] = gg
            dd = dd_t[g]
            nc.scalar.activation(
                gg[:, :n], dd[:, :n],
                mybir.ActivationFunctionType.Derivative_Erf,
                bias=0.0, scale=SDERF / Q,
            )

        def gg_at(b, off):
            """(tile, col) holding band element `off` of sample b."""
            for g, pos, lo, hi in seg_map[b]:
                if lo <= off < hi:
                    return gg_t[g], pos + off - lo
            raise AssertionError((b, off))

        def emit_img(g):
            for b in GROUPS[g]["done"]:
                img = img_pool.tile([W, W], f32, tag="img")
                img_t[b] = img
                for c in range(CHUNKS):
                    gx, ox = gg_at(b, 2 * CW * c)
                    gy, oy = gg_at(b, 2 * CW * c + CW)
                    nc.tensor.matmul(
                        img[:],
                        lhsT=gx[:, ox : ox + W],
                        rhs=gy[:, oy : oy + W],
                        start=(c == 0),
                        stop=(c == CHUNKS - 1),
                    )

        def emit_min_store(g):
            done = GROUPS[g]["done"]
            if not done:
                return
            n = len(done)
            outp = out_pool.tile([W, 2 * W], f32, tag="op")
            for k, b in enumerate(done):
                nc.vector.tensor_scalar(
                    outp[:, W * k : W * (k + 1)],
                    img_t[b][:],
                    DERF_FIX, 1.0,
                    mybir.AluOpType.mult, mybir.AluOpType.min,
                )
            nc.sync.dma_start(
                y_t.ap()[done[0] : done[0] + n].rearrange("b i j -> i b j"),
                outp[:, : W * n].rearrange("i (b j) -> i b j", b=n),
            )

        # -------- software-pipelined emission --------
        ensure_rmm(0)
        ensure_rmm(1)
        ensure_rmm(2)
        emit_sub(0)
        for g in range(NG):
            ensure_rmm(g + 3)
            if g + 1 < NG:
                emit_sub(g + 1)
            emit_act(g)
            emit_img(g)
            if g >= 1:
                emit_min_store(g - 1)
        emit_min_store(NG - 1)

    if legalize:
        _legalize_waits(nc)
    return nc


_PROGRAM = None


def kernel(x: np.ndarray, _trace: bool = False) -> np.ndarray:
    global _PROGRAM, LAST_RESULTS
    assert x.shape == (B, L, 8) and x.dtype == np.float32, (x.shape, x.dtype)
    if _PROGRAM is None:
        _PROGRAM = build_program()
    nc = _PROGRAM
    shards = np.split(np.ascontiguousarray(x), NCORES, axis=0)
    in_maps = [{"x": _host_ctrl(s)} for s in shards]
    res = run_bass_kernel_spmd(nc, in_maps, list(range(NCORES)), trace=_trace)
    LAST_RESULTS = res
    return np.concatenate([res.results[i]["y"] for i in range(NCORES)], axis=0)
